# revision 11
# baseline (speedup 1.0000x reference)
"""AGD loss (angular-Gaussian density contrastive loss) on 8 TRN2 NeuronCores.

Math: the reference evaluates, per column j (n = V*B = 32768 columns) and per
class c (C = 100), the Saw-series density s(y[c,j]) where
    s(a) = sum_n c_n a^n,   c_n = 2^{n/2} Gamma((d+n)/2) / (Gamma(d/2) n!)
(the huge exp(log_Cd - 1/(2 sigma^2)) prefactor cancels in the final
log-ratio).  The coefficients c_n * n! are exactly the raw moments of a
chi(d=128) random variable R, so

    s(a) = E_R[exp(R * a)]  (40-term truncation is far below fp32 noise)

and an M-point Gauss quadrature on the chi(128) weight collapses the 40-term
polynomial to M=4 exponentials:

    s(a) ~= sum_m exp(r_m * a + ln w_m)      (max rel err ~3e-7 on |a|<=0.55)

On-device per core (data-parallel over columns, 4096 columns/core):
  - input tile X [101, 4096] fp16: rows 0..99 = y classes, row 100 = the
    host-gathered own-class value y[label_j, j]
  - ScalarE: M activation(Exp, scale=r_m, bias=ln w_m) passes (fp16 out)
  - TensorE: [101->2] ones/one-hot matmul accumulating the M terms in PSUM:
    row 0 = norms_j = sum_c s(y[c,j]),  row 1 = s_lab_j
  - ScalarE: Ln over PSUM with accum_out -> per-chunk partial sums [2,1]
  - host: loss = sum(log norms) - sum(log s_lab), summed in float64
"""

import numpy as np

import concourse.bass as bass
import concourse.bacc as bacc
import concourse.mybir as mybir
from concourse.tile import TileContext
from concourse.bass_utils import run_bass_kernel_spmd

N_CORES = 8
B = 16384
V = 2
D = 128
C = 100
N = V * B                 # 32768 columns
NLOC = N // N_CORES       # 4096 columns per core
P = C + 1                 # 100 class rows + 1 own-class row
FC = 2048                 # columns per chunk
NCHUNK = NLOC // FC
MM_N = 512                # matmul moving free dim
NACC = NLOC // MM_N       # one Ln + accum slot per PSUM bank

# Gauss quadrature (M=4) for the chi(128) MGF: s(a) ~= sum exp(R[m]*a + LNW[m])
QR = [9.728271574810769, 10.815810838856631, 11.864472056285917, 13.024152044225334]
QLNW = [-2.906453165571208, -0.7384946275418353, -0.8470723938165576, -3.248563367872379]
M = len(QR)

IN_DT = mybir.dt.float16

_CACHE = {}
LAST_RESULT = None  # BassKernelResults of the most recent run (for profiling)
TRACE = False


def build_bass():
    nc = bacc.Bacc(None, target_bir_lowering=False)
    x = nc.declare_dram_parameter("x", [P, NLOC], IN_DT, isOutput=False)
    sel_in = nc.declare_dram_parameter("sel", [P, 2], IN_DT, isOutput=False)
    out = nc.declare_dram_parameter("out", [2, NACC], mybir.dt.float32, isOutput=True)

    with TileContext(nc) as tc:
        with (
            tc.tile_pool(name="const", bufs=1) as cpool,
            tc.tile_pool(name="xin", bufs=2) as xpool,
            tc.tile_pool(name="exp", bufs=3) as epool,
            tc.tile_pool(name="ln", bufs=1) as lpool,
            tc.tile_pool(name="acc", bufs=1) as apool,
            tc.tile_pool(name="ps", bufs=1, space="PSUM") as ppool,
        ):
            # selection matrix: col 0 sums the 100 class rows (norms),
            # col 1 picks row 100 (own-class density)
            sel = cpool.tile([P, 2], IN_DT)
            nc.sync.dma_start(sel[:, :], sel_in[:, :])

            # per-partition bias vectors for the Exp activations (ln w_m)
            biases = cpool.tile([P, M], mybir.dt.float32)
            for m in range(M):
                nc.vector.memset(biases[:, m : m + 1], QLNW[m])

            acc = apool.tile([2, NACC], mybir.dt.float32)
            ps = ppool.tile([2, NLOC], mybir.dt.float32)

            for k in range(NCHUNK):
                xt = xpool.tile([P, FC], IN_DT)
                nc.sync.dma_start(xt[:, :], x[:, k * FC : (k + 1) * FC])
                for m in range(M):
                    et = epool.tile([P, FC], IN_DT)
                    nc.scalar.activation(
                        et[:, :],
                        xt[:, :],
                        mybir.ActivationFunctionType.Exp,
                        bias=biases[:, m : m + 1],
                        scale=QR[m],
                    )
                    for b in range(FC // MM_N):
                        col = k * FC + b * MM_N
                        nc.tensor.matmul(
                            ps[:, col : col + MM_N],
                            sel[:, :],
                            et[:, b * MM_N : (b + 1) * MM_N],
                            start=(m == 0),
                            stop=(m == M - 1),
                        )

            lt = lpool.tile([2, NLOC], mybir.dt.float32)
            for k in range(NACC):
                nc.scalar.activation(
                    lt[:, k * MM_N : (k + 1) * MM_N],
                    ps[:, k * MM_N : (k + 1) * MM_N],
                    mybir.ActivationFunctionType.Ln,
                    accum_out=acc[:, k : k + 1],
                )
            nc.sync.dma_start(out[:, :], acc[:, :])

    nc.finalize()
    return nc


def _get_nc():
    if "nc" not in _CACHE:
        _CACHE["nc"] = build_bass()
    return _CACHE["nc"]


def kernel(features: np.ndarray, labels: np.ndarray) -> np.ndarray:
    global LAST_RESULT
    features = np.asarray(features)
    labels = np.asarray(labels)

    # view-major flatten: [B, V, D] -> [V*B, D]
    feats = np.ascontiguousarray(features.transpose(1, 0, 2).reshape(N, D))
    labels_rep = np.tile(labels.astype(np.int64), V)
    alab = feats[np.arange(N), labels_rep]  # own-class coordinate per column

    sel_np = np.zeros((P, 2), dtype=np.float16)
    sel_np[:C, 0] = 1.0
    sel_np[C, 1] = 1.0

    in_maps = []
    for i in range(N_CORES):
        sl = slice(i * NLOC, (i + 1) * NLOC)
        X = np.empty((P, NLOC), dtype=np.float16)
        X[:C, :] = feats[sl, :C].T
        X[C, :] = alab[sl]
        in_maps.append({"x": X, "sel": sel_np})

    nc = _get_nc()
    res = run_bass_kernel_spmd(nc, in_maps, list(range(N_CORES)), trace=TRACE)
    LAST_RESULT = res

    total = np.float64(0.0)
    for i in range(N_CORES):
        o = res.results[i]["out"].astype(np.float64)
        total += o[0].sum() - o[1].sum()
    return np.asarray(total, dtype=np.float64)
